# revision 1
# baseline (speedup 1.0000x reference)
"""Multi-head causal attention (B=2, S=2048, H=1024, 16 heads x 64, RoPE) on 8 trn2 cores.

Sharding: tensor-parallel over heads (2 heads/core) for QKV+attention, then a
per-batch AllToAll switches to token-parallel for the output projection. Each
core owns 256 tokens of each batch; the host concatenates disjoint row slices.

Design (per core c, heads h0=2c, h0+1):
 - xT [1024, 4096] feature-major activations, host-transposed + bf16-cast, so
   both qT/kT (feature-major, via lhsT=W.T rhs=xT) and V (token-major, via a
   small PE transpose of vT) come straight out of matmuls - no activation
   transposes on device.
 - RoPE applied feature-major with host cos/sin tables; the half-rotation
   partition swap is an SBUF->SBUF DMA, sin-multiply on GpSimd, cos-multiply
   and add on DVE. qT/kT stored float32r (full PE rate at N>=512).
 - Scores computed TRANSPOSED: sT[k, q] = matmul(lhsT=kT_block, rhs=qT_chunk)
   with the two heads packed in row groups (0,0)/(64,0) -> softmax probs come
   out in the [k, q] layout PV wants; no probability transposes. Softmax max-
   subtraction is skipped (logits ~N(0,1); exp is safe in fp32). Causal mask =
   bf16 0/1 multiply on the diagonal block's probs.
 - exp on ACT writes bf16 probs; PV matmuls (lhsT=V[tok,d], rhs=probsT) pack
   the heads in col groups (0,0)/(0,64); per-q softmax sums accumulate in the
   same pass via 0/1 selector-column matmuls into PSUM rows 0/33.
 - Normalization: sums broadcast across partitions with a K=34 selector
   matmul, reciprocal on DVE, then fused into the PSUM->SBUF context copy.
 - Per-batch AllToAll on [8, 128, 256] bf16 ctxT blocks: batch 0'''s collective
   and out-proj overlap batch 1'''s attention. Received buffer is exactly
   ctx_shard.T = lhsT of the out-proj (x W_out.T, fp32 out).
"""

import numpy as np

import concourse.bacc as bacc
import concourse.mybir as mybir
import concourse.tile as tile
from concourse.bass_utils import run_bass_kernel_spmd

F32 = mybir.dt.float32
F32R = mybir.dt.float32r
BF16 = mybir.dt.bfloat16
EXP = mybir.ActivationFunctionType.Exp

B, S, H = 2, 2048, 1024
NH, HD = 16, 64
NCORES = 8
T = B * S            # 4096 flattened tokens (b-major)
TBLK = T // NCORES   # 512 tokens per a2a block
P = 128


def _build_nc():
    nc = bacc.Bacc(None, num_devices=NCORES)

    xT_d = nc.dram_tensor("xT", [H, T], BF16, kind="ExternalInput")
    wqkvT_d = nc.dram_tensor("wqkvT", [H, 384], BF16, kind="ExternalInput")
    woutT_d = nc.dram_tensor("woutT", [H, H], BF16, kind="ExternalInput")
    costab_d = nc.dram_tensor("costab", [P, S], F32, kind="ExternalInput")
    sintab_d = nc.dram_tensor("sintab", [P, S], F32, kind="ExternalInput")
    maskT_d = nc.dram_tensor("maskT", [P, P], BF16, kind="ExternalInput")
    identf_d = nc.dram_tensor("identf", [P, P], BF16, kind="ExternalInput")
    esel_d = nc.dram_tensor("esel", [P, 4], BF16, kind="ExternalInput")
    bsel_d = nc.dram_tensor("bsel", [34, P], F32, kind="ExternalInput")
    out_d = nc.dram_tensor("out", [TBLK, H], F32, kind="ExternalOutput")

    with tile.TileContext(nc) as tc:
        with (
            tc.tile_pool(name="long", bufs=1) as lp,
            tc.tile_pool(name="dram", bufs=1, space="DRAM") as dp,
        ):
            # long-lived tiles
            qT = [lp.tile([P, S], F32R, tag=f"qT{b}", name=f"qT{b}") for b in range(B)]
            kT = [lp.tile([P, S], F32R, tag=f"kT{b}", name=f"kT{b}") for b in range(B)]
            V = [[lp.tile([P, 16, HD], BF16, tag=f"V{b}{h}", name=f"V{b}{h}") for h in range(2)]
                 for b in range(B)]
            ctxT = [lp.tile([P, S], BF16, tag=f"ctxT{b}", name=f"ctxT{b}") for b in range(B)]
            maskT_t = lp.tile([P, P], BF16, tag="maskT")
            identf_t = lp.tile([P, P], BF16, tag="identf")
            esel_t = lp.tile([P, 4], BF16, tag="esel")
            bsel_t = lp.tile([34, P], F32R, tag="bsel")
            wo = lp.tile([P, 8, H], BF16, tag="wo")

            nc.sync.dma_start(maskT_t[:], maskT_d[:])
            nc.sync.dma_start(identf_t[:], identf_d[:])
            nc.sync.dma_start(esel_t[:], esel_d[:])
            nc.sync.dma_start(bsel_t[:], bsel_d[:].bitcast(F32R))

            a2a_in = [dp.tile([NCORES, P, 256], BF16, name=f"a2a_in{b}",
                              tag=f"a2a_in{b}") for b in range(B)]
            a2a_out = [dp.tile([NCORES, P, 256], BF16, name=f"a2a_out{b}",
                               tag=f"a2a_out{b}") for b in range(B)]

            # ---------------- Phase 1: QKV projection + RoPE + V transpose
            with (
                tc.tile_pool(name="p1c", bufs=1) as p1c,
                tc.tile_pool(name="p1s", bufs=3) as p1s,
                tc.tile_pool(name="p1v", bufs=2) as p1v,
                tc.tile_pool(name="p1t", bufs=4) as p1t,
                tc.tile_pool(name="ps1", bufs=6, space="PSUM") as ps1,
                tc.tile_pool(name="ps1t", bufs=2, space="PSUM") as ps1t,
            ):
                wq = p1c.tile([P, 8, 384], BF16, tag="wq")
                wqkv_r = wqkvT_d[:].rearrange("(k p) c -> p k c", p=P)
                nc.sync.dma_start(wq[:, 0:4, :], wqkv_r[:, 0:4, :])
                nc.sync.dma_start(wq[:, 4:8, :], wqkv_r[:, 4:8, :])
                costab_t = p1c.tile([P, S], F32, tag="costab")
                sintab_t = p1c.tile([P, S], F32, tag="sintab")

                # per 512-token chunk: 24 matmuls (q,k,v x 8 k-tiles)
                vts = {}  # pending v-chunk sbuf tiles for transposes
                for b in range(B):
                    for ch in range(4):
                        tok0 = b * S + ch * 512
                        c0 = ch * 512
                        xt_r = (xT_d[:, tok0:tok0 + 512]
                                .rearrange("(k p) t -> p k t", p=P))
                        xta = p1s.tile([P, 4, 512], BF16, tag="xta")
                        xtb = p1s.tile([P, 4, 512], BF16, tag="xtb")
                        nc.sync.dma_start(xta[:], xt_r[:, 0:4, :])
                        nc.sync.dma_start(xtb[:], xt_r[:, 4:8, :])
                        if b == 0 and ch == 0:
                            nc.sync.dma_start(costab_t[:], costab_d[:])
                            nc.sync.dma_start(sintab_t[:], sintab_d[:])
                        for m in range(3):
                            ps = ps1.tile([P, 512], F32, tag="qkv_ps")
                            for kt in range(8):
                                xt_half = xta if kt < 4 else xtb
                                nc.tensor.matmul(
                                    ps[:],
                                    wq[:, kt, m * P:(m + 1) * P],
                                    xt_half[:, kt % 4, :],
                                    start=(kt == 0), stop=(kt == 7),
                                )
                            if m < 2:
                                tgt = qT[b] if m == 0 else kT[b]
                                nc.vector.tensor_copy(tgt[:, c0:c0 + 512], ps[:])
                                swp = p1t.tile([P, 512], F32, tag="swp")
                                for g in range(4):
                                    dst = g * 32
                                    srcp = dst ^ 32
                                    nc.scalar.dma_start(
                                        swp[dst:dst + 32, :],
                                        tgt[srcp:srcp + 32, c0:c0 + 512]
                                        .bitcast(F32),
                                    )
                                nc.gpsimd.tensor_mul(
                                    swp[:], swp[:], sintab_t[:, c0:c0 + 512])
                                nc.vector.tensor_mul(
                                    tgt[:, c0:c0 + 512],
                                    tgt[:, c0:c0 + 512].bitcast(F32),
                                    costab_t[:, c0:c0 + 512])
                                nc.vector.tensor_add(
                                    tgt[:, c0:c0 + 512],
                                    tgt[:, c0:c0 + 512].bitcast(F32),
                                    swp[:])
                            else:
                                vt = p1v.tile([P, 512], BF16, tag="vT")
                                nc.scalar.copy(vt[:], ps[:])
                                vts[(b, ch)] = vt
                        # emit previous chunk's V transposes (software pipeline:
                        # keeps PE from stalling on the ACT copy)
                        for key in list(vts):
                            if key != (b, ch):
                                _v_transposes(nc, ps1t, V, identf_t, vts.pop(key), key)
                for key in list(vts):
                    _v_transposes(nc, ps1t, V, identf_t, vts.pop(key), key)

            nc.sync.dma_start(
                wo[:], woutT_d[:].rearrange("(j p) n -> p j n", p=P)
            )

            # ---------------- Phase 2: attention, transposed softmax
            with (
                tc.tile_pool(name="p2", bufs=4) as p2,
                tc.tile_pool(name="p2n", bufs=2) as p2n,
                tc.tile_pool(name="ps2s", bufs=4, space="PSUM") as ps2s,
                tc.tile_pool(name="ps2c", bufs=2, space="PSUM") as ps2c,
                tc.tile_pool(name="ps2m", bufs=2, space="PSUM") as ps2m,
                tc.tile_pool(name="ps2b", bufs=1, space="PSUM") as ps2b,
                tc.tile_pool(name="p3", bufs=2) as p3,
            ):
                ctxs0 = None
                sc_tiles = []
                for i_ in range(2):
                    sct = p2n.tile([34, 512], F32R, tag=f"sumrow{i_}",
                                   name=f"sc{i_}", bufs=1)
                    # rows 2:32 feed the K=34 broadcast matmul with zero
                    # weights -- zero once so they're finite (0 x NaN = NaN)
                    nc.vector.memset(sct[0:32, :].bitcast(F32), 0.0)
                    sc_tiles.append(sct)
                for b in range(B):
                    for qs in (3, 2, 1, 0):
                        pctx = ps2c.tile([P, 512], F32, tag="ctx")
                        psums = ps2m.tile([34, 512], F32, tag="sums",
                                          name="sums", bufs=1)
                        sc = sc_tiles[(b * 4 + qs) % 2]
                        nkb = 4 * qs + 4
                        pend = []  # pipelined PV work: (kb, h, probs, qoff, N)
                        for kb in range(nkb):
                            j = kb - 4 * qs
                            qoff = max(0, j) * P
                            N = 512 - qoff
                            for h in range(2):
                                psT = ps2s.tile([P, 512], F32, tag="sT")
                                nc.tensor.matmul(
                                    psT[:, 0:N],
                                    kT[b][h * HD:(h + 1) * HD, kb * P:(kb + 1) * P],
                                    qT[b][h * HD:(h + 1) * HD,
                                          qs * 512 + qoff:(qs + 1) * 512],
                                    start=True, stop=True,
                                    tile_position=(h * HD, 0),
                                )
                                pb = p2.tile([P, 512], BF16, tag="probs",
                                             bufs=10)
                                nc.scalar.activation(
                                    pb[:, 0:N], psT[:, 0:N], EXP, scale=0.125)
                                if j >= 0:
                                    # zero the strictly-upper triangle of the
                                    # diagonal 128-block (bf16 binary mask)
                                    nc.vector.tensor_mul(
                                        pb[:, 0:P], pb[:, 0:P], maskT_t[:])
                                pend.append((kb, h, pb, qoff, N))
                            # emit PV/sums one kb behind the score matmuls
                            while len(pend) > 8:
                                _pv_sums(nc, pctx, psums, V, esel_t, b, qs,
                                         nkb, pend.pop(0))
                        while pend:
                            _pv_sums(nc, pctx, psums, V, esel_t, b, qs, nkb,
                                     pend.pop(0))
                        # normalize: broadcast per-q sums, reciprocal, fused copy
                        nc.scalar.copy(sc[0:2, :], psums[0:2, :])
                        nc.scalar.copy(sc[32:34, :], psums[32:34, :])
                        # broadcast per-q sums across partitions: rows 0:64 get
                        # head0 sums, rows 64:128 head1 (K=34 0/1 selector,
                        # zero rows in the gap contribute nothing)
                        pbc = ps2b.tile([P, 512], F32, tag="bc")
                        nc.tensor.matmul(
                            pbc[:], bsel_t[0:34, :], sc[0:34, :],
                            start=True, stop=True)
                        rb = p2n.tile([P, 512], F32, tag="recip")
                        nc.vector.reciprocal(rb[:], pbc[:])
                        nc.vector.tensor_mul(
                            ctxT[b][:, qs * 512:(qs + 1) * 512], pctx[:], rb[:])
                        for half in range(2):
                            blk = 2 * qs + half
                            o0 = qs * 512 + half * 256
                            nc.scalar.dma_start(
                                a2a_in[b][blk], ctxT[b][:, o0:o0 + 256])
                    if b == 1:
                        # load + out-proj batch 0 BEFORE collective #1 so its
                        # PE/DMA work isn't queue-ordered behind it
                        ctxs0 = []
                        for jj in range(8):
                            cj = p3.tile([P, 256], BF16, tag=f"ctxs{jj}",
                                         name=f"ctxs0{jj}", bufs=2)
                            nc.sync.dma_start(cj[:], a2a_out[0][jj])
                            ctxs0.append(cj)
                        _out_proj(nc, tc, p3, ps2s, ctxs0, wo, out_d, 0)
                    # batch b's AllToAll overlaps batch b+1's attention
                    nc.gpsimd.collective_compute(
                        "AllToAll",
                        mybir.AluOpType.bypass,
                        replica_groups=[list(range(NCORES))],
                        ins=[a2a_in[b].opt()],
                        outs=[a2a_out[b].opt()],
                    )
                    if b == 0:
                        pass
                    else:
                        ctxs1 = []
                        for jj in range(8):
                            cj = p3.tile([P, 256], BF16, tag=f"ctxs{jj}",
                                         name=f"ctxs1{jj}", bufs=2)
                            nc.sync.dma_start(cj[:], a2a_out[1][jj])
                            ctxs1.append(cj)
                        _out_proj(nc, tc, p3, ps2s, ctxs1, wo, out_d, 1)

    nc.finalize()
    return nc


def _out_proj(nc, tc, p3, ps_pool, ctxs, wo, out_d, bb):
    for mt in range(2):
        for nt in range(2):
            po = ps_pool.tile([P, 512], F32, tag="sT", name="po")
            for jj in range(8):
                nc.tensor.matmul(
                    po[:],
                    ctxs[jj][:, mt * P:(mt + 1) * P],
                    wo[:, jj, nt * 512:(nt + 1) * 512],
                    start=(jj == 0), stop=(jj == 7),
                )
            ob = p3.tile([P, 512], F32, tag="ob", name="ob")
            nc.scalar.copy(ob[:], po[:])
            nc.scalar.dma_start(
                out_d[bb * 256 + mt * P:bb * 256 + (mt + 1) * P,
                      nt * 512:(nt + 1) * 512],
                ob[:])


def _v_transposes(nc, ps_pool, V, identf_t, vt, key):
    b, ch = key
    for h in range(2):
        for tb in range(4):
            pst = ps_pool.tile([P, HD], BF16, tag="vt_ps", name="vt_ps")
            nc.tensor.transpose(
                pst[:],
                vt[h * HD:(h + 1) * HD, tb * P:(tb + 1) * P],
                identf_t[h * HD:(h + 1) * HD, h * HD:(h + 1) * HD],
            )
            gb = ch * 4 + tb
            nc.vector.tensor_copy(V[b][h][:, gb, :], pst[:])


def _pv_sums(nc, pctx, psums, V, esel_t, b, qs, nkb, item):
    kb, h, pb, qoff, N = item
    nc.tensor.matmul(
        pctx[h * HD:(h + 1) * HD, qoff:512],
        V[b][h][:, kb, :],
        pb[:, 0:N],
        start=(kb == 0), stop=(kb == nkb - 1),
        tile_position=(0, h * HD),
    )
    nc.tensor.matmul(
        psums[h * 32:h * 32 + 2, qoff:512],
        esel_t[:, 2 * h:2 * h + 2],
        pb[:, 0:N],
        start=(kb == 0), stop=(kb == nkb - 1),
        tile_position=(0, h * 32),
        skip_group_check=True,
    )


_NC_CACHE = None


def _get_nc():
    global _NC_CACHE
    if _NC_CACHE is None:
        _NC_CACHE = _build_nc()
    return _NC_CACHE


def _host_tables():
    j = np.arange(32)
    inv = (10000.0 ** (-(j.astype(np.float64)) / 32.0))
    pos = np.arange(S, dtype=np.float64)
    fr = pos[:, None] * inv[None, :]              # [S, 32]
    cosT = np.cos(fr).T.astype(np.float32)        # [32, S]
    sinT = np.sin(fr).T.astype(np.float32)
    costab = np.tile(cosT, (4, 1))                # [128, S]
    sintab = np.concatenate([-sinT, sinT, -sinT, sinT], 0)
    import ml_dtypes
    kk = np.arange(P)[:, None]
    qq = np.arange(P)[None, :]
    maskT = np.where(kk <= qq, 1.0, 0.0).astype(ml_dtypes.bfloat16)
    identf = np.eye(P, dtype=np.float32).astype(ml_dtypes.bfloat16)
    return costab, sintab, maskT, identf


def _selectors():
    import ml_dtypes
    esel = np.zeros((P, 4), dtype=np.float32)
    esel[:, 0] = 1.0  # head0 sums -> psum row 0
    esel[:, 3] = 1.0  # head1 sums -> psum row 1
    esel = esel.astype(ml_dtypes.bfloat16)
    bsel = np.zeros((34, P), dtype=np.float32)
    bsel[0, 0:64] = 1.0    # head0 sums (psum row 0)
    bsel[33, 64:128] = 1.0  # head1 sums (psum row 33)
    return esel, bsel


def _make_in_maps(x, W_qkv, W_out):
    import ml_dtypes
    costab, sintab, maskT, identf = _host_tables()
    esel, bsel = _selectors()
    xT = np.ascontiguousarray(x.reshape(T, H).T).astype(ml_dtypes.bfloat16)
    woutT = np.ascontiguousarray(W_out.T).astype(ml_dtypes.bfloat16)
    in_maps = []
    for c in range(NCORES):
        h0 = 2 * c
        rows = np.concatenate([
            W_qkv[HD * h0:HD * (h0 + 2)],
            W_qkv[H + HD * h0:H + HD * (h0 + 2)],
            W_qkv[2 * H + HD * h0:2 * H + HD * (h0 + 2)],
        ], axis=0)                                        # [384, H]
        wqkvT = np.ascontiguousarray(rows.T).astype(ml_dtypes.bfloat16)
        in_maps.append({
            "xT": xT, "wqkvT": wqkvT, "woutT": woutT,
            "costab": costab, "sintab": sintab,
            "maskT": maskT, "identf": identf,
            "esel": esel, "bsel": bsel,
        })
    return in_maps


def _run_spmd(x, W_qkv, W_out, **kw):
    nc = _get_nc()
    in_maps = _make_in_maps(x, W_qkv, W_out)
    return run_bass_kernel_spmd(nc, in_maps, core_ids=list(range(NCORES)),
                                **kw)


def kernel(x, W_qkv, W_out):
    x = np.asarray(x, dtype=np.float32)
    W_qkv = np.asarray(W_qkv, dtype=np.float32)
    W_out = np.asarray(W_out, dtype=np.float32)
    res = _run_spmd(x, W_qkv, W_out)
    # core c owns tokens [c*256,(c+1)*256) of each batch (flattened b-major)
    full = np.empty((T, H), dtype=np.float32)
    for c in range(NCORES):
        o = res.results[c]["out"]
        full[c * 256:(c + 1) * 256] = o[0:256]
        full[S + c * 256:S + (c + 1) * 256] = o[256:512]
    return full.reshape(B, S, H)



# revision 21
# speedup vs baseline: 1.1207x; 1.1207x over previous
"""Multi-head causal attention (B=2, S=2048, H=1024, 16 heads x 64, RoPE) on 8 trn2 cores.

Sharding: tensor-parallel over heads (2 heads/core) for QKV+attention, then
AllToAlls switch to token-parallel for the output projection. Each core owns
4x128-token slices (one per batch-half); the host concatenates row slices.

Key structure (per core c, heads h0=2c, h0+1):
 - xT [1024, 4096] feature-major bf16 activations (host-transposed), one DMA
   per 512-token chunk. QKV = 24 matmuls/chunk from a [128, 8, 384] W tile.
 - RoPE feature-major in bf16 (2x DVE modes) with host cos/sin tables; the
   half-rotation partition swap is 4 SBUF->SBUF DMAs from a scratch tile
   (no WAR hazard), sin-mul on GpSimd, cos-mul + add on DVE.
 - Scores TRANSPOSED in bf16: sT[k, q] = matmul(lhsT=kT_blk, rhs=qT_chunk),
   both heads in one [128, 2, 512] PSUM tile -> ONE merged exp per k-block.
   Softmax max-subtraction skipped (logits ~N(0,1)). Causal mask = bf16 0/1
   multiply on the diagonal block.
 - PV FLIPPED: ctx[q, d] = matmul(lhsT=probsT[k, q-blk], rhs=[V | ones]).
   Cost follows the 65-wide free dim, and the ones column accumulates the
   softmax sums for free (column 64).
 - Normalize: sums are a per-partition scalar -> DVE reciprocal + GpSimd
   tensor_scalar multiplies; ctx -> ctxT via PE transpose + DVE copy.
 - FOUR collectives (one per batch-half, [8, 128, 128] bf16) so only the
   last 256KB AllToAll is exposed in the tail; out-projections for earlier
   halves run as soon as their collective lands.
 - Emission uses a filler pump: phase-1 QKV m-groups / V-transposes /
   out-projections are interleaved between attention k-block units so the
   PE never idles (and stays p-state ramped) while ACT streams exps.
"""

from collections import deque

import numpy as np

import concourse.bacc as bacc
import concourse.mybir as mybir
import concourse.tile as tile
from concourse.bass_utils import run_bass_kernel_spmd

F32 = mybir.dt.float32
BF16 = mybir.dt.bfloat16
EXP = mybir.ActivationFunctionType.Exp

B, S, H = 2, 2048, 1024
NH, HD = 16, 64
NCORES = 8
T = B * S            # 4096 flattened tokens (b-major)
TBLK = T // NCORES   # 512 tokens per core
P = 128


def _build_nc():
    nc = bacc.Bacc(None, num_devices=NCORES)

    xT_d = nc.dram_tensor("xT", [H, T], BF16, kind="ExternalInput")
    wqkvT_d = nc.dram_tensor("wqkvT", [H, 384], BF16, kind="ExternalInput")
    woutT_d = nc.dram_tensor("woutT", [H, H], BF16, kind="ExternalInput")
    costab_d = nc.dram_tensor("costab", [P, S], BF16, kind="ExternalInput")
    sintab_d = nc.dram_tensor("sintab", [P, S], BF16, kind="ExternalInput")
    maskT_d = nc.dram_tensor("maskT", [P, 2 * P], BF16, kind="ExternalInput")
    identf_d = nc.dram_tensor("identf", [P, P], BF16, kind="ExternalInput")
    out_d = nc.dram_tensor("out", [TBLK, H], F32, kind="ExternalOutput")

    with tile.TileContext(nc) as tc:
        with (
            tc.tile_pool(name="long", bufs=1) as lp,
            tc.tile_pool(name="dram", bufs=1, space="DRAM") as dp,
            tc.tile_pool(name="p1s", bufs=4) as p1s,
            tc.tile_pool(name="p1v", bufs=2) as p1v,
            tc.tile_pool(name="p1t", bufs=3) as p1t,
            tc.tile_pool(name="ps1", bufs=1, space="PSUM") as ps1,
            tc.tile_pool(name="ps1t", bufs=1, space="PSUM") as ps1t,
            tc.tile_pool(name="p2", bufs=8) as p2,
            tc.tile_pool(name="p2n", bufs=3) as p2n,
            tc.tile_pool(name="ps2s", bufs=2, space="PSUM") as ps2s,
            tc.tile_pool(name="ps2c", bufs=2, space="PSUM") as ps2c,
            tc.tile_pool(name="p3", bufs=2) as p3,
        ):
            # long-lived tiles
            qT = [lp.tile([P, S], BF16, tag=f"qT{b}", name=f"qT{b}") for b in range(B)]
            kT = [lp.tile([P, S], BF16, tag=f"kT{b}", name=f"kT{b}") for b in range(B)]
            V = [[lp.tile([P, 16, 65], BF16, tag=f"V{b}{h}", name=f"V{b}{h}")
                  for h in range(2)] for b in range(B)]
            ctxT = [lp.tile([P, S], BF16, tag=f"ctxT{b}", name=f"ctxT{b}")
                    for b in range(B)]
            maskT_t = lp.tile([P, 2, P], BF16, tag="maskT")
            identf_t = lp.tile([P, P], BF16, tag="identf")
            wo = lp.tile([P, 8, H], BF16, tag="wo")
            wq = lp.tile([P, 8, 384], BF16, tag="wq")
            costab_t = lp.tile([P, S], BF16, tag="costab")
            sintab_t = lp.tile([P, S], BF16, tag="sintab")

            nc.sync.dma_start(wq[:], wqkvT_d[:].rearrange("(k p) c -> p k c", p=P))

            def load_xt(b, ch):
                tok0 = b * S + ch * 512
                xt = p1s.tile([P, 8, 512], BF16, tag="xt", name=f"xt{b}{ch}")
                nc.sync.dma_start(
                    xt[:], xT_d[:, tok0:tok0 + 512]
                    .rearrange("(k p) t -> p k t", p=P))
                return xt

            xt00 = load_xt(0, 0)
            nc.sync.dma_start(
                maskT_t[:], maskT_d[:].rearrange("p (h k) -> p h k", h=2))
            nc.sync.dma_start(identf_t[:], identf_d[:])
            nc.sync.dma_start(costab_t[:], costab_d[:])
            nc.sync.dma_start(sintab_t[:], sintab_d[:])
            for b in range(B):
                for h in range(2):
                    nc.vector.memset(V[b][h][:, :, 64:65], 1.0)

            a2a_in = [[dp.tile([NCORES, P, P], BF16, name=f"a2a_in{b}{f}",
                               tag=f"a2a_in{b}{f}") for f in range(2)]
                      for b in range(B)]
            a2a_out = [[dp.tile([NCORES, P, P], BF16, name=f"a2a_out{b}{f}",
                                tag=f"a2a_out{b}{f}") for f in range(2)]
                       for b in range(B)]

            def qkv_m(b, ch, m, xt, st):
                """One QKV output-tile: 8 matmuls + RoPE or V staging.
                q and k stage into one [P, 2, 512] tmp so the partition-swap
                is 4 chunk-level DMAs instead of 8."""
                c0 = ch * 512
                ps = ps1.tile([P, 512], F32, tag="qkv_ps")
                for kt in range(8):
                    nc.tensor.matmul(
                        ps[:], wq[:, kt, m * P:(m + 1) * P], xt[:, kt, :],
                        start=(kt == 0), stop=(kt == 7),
                    )
                if m < 2:
                    if m == 0:
                        st["tmp"] = p1t.tile([P, 2, 512], BF16, tag="tmp", name="tmp")
                        st["swp"] = p1t.tile([P, 2, 512], BF16, tag="swp", name="swp")
                    nc.vector.tensor_copy(st["tmp"][:, m, :], ps[:])
                    if m == 1:
                        tmp, swp = st["tmp"], st["swp"]
                        for g in range(4):
                            dst = g * 32
                            srcp = dst ^ 32
                            nc.sync.dma_start(swp[dst:dst + 32],
                                              tmp[srcp:srcp + 32])
                        for mm, tgt in ((0, qT[b]), (1, kT[b])):
                            nc.gpsimd.tensor_mul(
                                swp[:, mm, :], swp[:, mm, :],
                                sintab_t[:, c0:c0 + 512])
                            nc.vector.tensor_mul(
                                tgt[:, c0:c0 + 512], tmp[:, mm, :],
                                costab_t[:, c0:c0 + 512])
                            nc.vector.tensor_add(
                                tgt[:, c0:c0 + 512], tgt[:, c0:c0 + 512],
                                swp[:, mm, :])
                    return None
                vt = p1v.tile([P, 512], BF16, tag="vT", name=f"vT{b}{ch}")
                nc.vector.tensor_copy(vt[:], ps[:])
                return vt

            def v_transposes(b, ch, vt):
                for h in range(2):
                    pst = ps1t.tile([P, 256], BF16, tag="tp", name="vt_ps")
                    for tb in range(4):
                        nc.tensor.transpose(
                            pst[:, tb * HD:(tb + 1) * HD],
                            vt[h * HD:(h + 1) * HD, tb * P:(tb + 1) * P],
                            identf_t[h * HD:(h + 1) * HD, h * HD:(h + 1) * HD],
                        )
                    nc.vector.tensor_copy(
                        V[b][h][:, ch * 4:(ch + 1) * 4, 0:HD],
                        pst[:].rearrange("p (g d) -> p g d", g=4))

            xts = {}

            def p1_units(b, ch):
                """Filler units for one 512-token chunk of QKV+RoPE+V
                (the xt load for (b, ch) must be emitted beforehand)."""
                state = {"vt": None, "st": {}}

                def u_m(m):
                    r = qkv_m(b, ch, m, xts[(b, ch)], state["st"])
                    if r is not None:
                        state["vt"] = r

                def u_vt():
                    v_transposes(b, ch, state["vt"])

                units = [lambda m=m: u_m(m) for m in range(3)]
                units.append(u_vt)
                return units

            fillers = deque()

            def pump():
                if fillers:
                    fillers.popleft()()

            def drain(n=None):
                cnt = len(fillers) if n is None else n
                for _ in range(cnt):
                    pump()

            def _pv_group(pctx, b, qs, qb, pbs_all):
                """All of query-block qb's PV accumulation as one contiguous
                PSUM group (banks allow only one open group at a time)."""
                last = 4 * qs + qb
                for h in range(2):
                    for kb in range(last + 1):
                        nc.tensor.matmul(
                            pctx[:, 2 * qb + h, 0:65],
                            pbs_all[kb][h][:, qb * P:(qb + 1) * P],
                            V[b][h][:, kb, :],
                            start=(kb == 0), stop=(kb == last),
                            skip_group_check=True,
                        )

            def p2_qs(b, qs):
                """Attention for one 512-query chunk: scores, exp, flipped PV,
                normalize, transpose back to ctxT. Pumps one filler unit per
                k-block to keep the PE busy while ACT runs the exps."""
                nkb = 4 * qs + 4
                pctx = ps2c.tile([P, 8, P], F32, tag="ctx", name="pctx")
                rb = p2n.tile([P, 4, 2, 1], F32, tag="recip")

                def normalize(qb):
                    # region qb of pctx just received its last accumulation
                    nc.vector.reciprocal(
                        rb[:, qb, :, :], pctx[:, 2 * qb:2 * qb + 2, 64:65])
                    cs = p2n.tile([P, 2, HD], BF16, tag="csb", bufs=4)
                    for h in range(2):
                        nc.vector.tensor_scalar_mul(
                            cs[:, h, :], pctx[:, 2 * qb + h, 0:HD],
                            rb[:, qb, h, 0:1])
                    pt = ps1t.tile([P, 256], BF16, tag="tp", name="ctxt_ps")
                    nc.tensor.transpose(pt[:, 0:P], cs[:], identf_t[:])
                    q0 = qs * 512 + qb * P
                    nc.vector.tensor_copy(ctxT[b][:, q0:q0 + P], pt[:, 0:P])

                pbs_all = []
                for kb in range(nkb):
                    j = kb - 4 * qs
                    qoff = max(0, j) * P
                    pbs = []
                    for h in range(2):
                        psT = ps2s.tile([P, 512], F32, tag="sT")
                        nc.tensor.matmul(
                            psT[:, qoff:512],
                            kT[b][h * HD:(h + 1) * HD, kb * P:(kb + 1) * P],
                            qT[b][h * HD:(h + 1) * HD,
                                  qs * 512 + qoff:(qs + 1) * 512],
                            start=True, stop=True,
                            tile_position=(h * HD, 0),
                        )
                        pb = p2.tile([P, 512], BF16, tag="probs", bufs=36)
                        nc.scalar.activation(
                            pb[:, qoff:512], psT[:, qoff:512], EXP,
                            scale=0.125)
                        if j >= 0:
                            nc.vector.tensor_mul(
                                pb[:, qoff:qoff + P],
                                pb[:, qoff:qoff + P],
                                maskT_t[:, h, :])
                        pbs.append(pb)
                    pbs_all.append(pbs)
                    pump()
                    if j >= 0:
                        # query-block j's last k-block just got its probs:
                        # emit its full PV accumulation + normalize
                        _pv_group(pctx, b, qs, j, pbs_all)
                        normalize(j)
                if qs % 2 == 1:
                    half = qs // 2
                    nc.sync.dma_start(
                        a2a_in[b][half][:].rearrange("g p t -> p g t"),
                        ctxT[b][:, half * 1024:(half + 1) * 1024]
                        .rearrange("p (g t) -> p g t", g=8))
                    nc.gpsimd.collective_compute(
                        "AllToAll",
                        mybir.AluOpType.bypass,
                        replica_groups=[list(range(NCORES))],
                        ins=[a2a_in[b][half].opt()],
                        outs=[a2a_out[b][half].opt()],
                    )

            ctxs_t = {}

            def ctxs_load(bb, half):
                ctxs = p3.tile([P, 8, P], BF16, tag="ctxs",
                               name=f"ctxs{bb}{half}")
                nc.sync.dma_start(
                    ctxs[:], a2a_out[bb][half][:].rearrange("j p t -> p j t"))
                ctxs_t[(bb, half)] = ctxs

            def out_proj(bb, half):
                """Out-projection for this core's 128 tokens of one
                batch-half; lands in out_d rows [bb*256+half*128, +128)."""
                ctxs = ctxs_t[(bb, half)]
                r0 = bb * 256 + half * P
                for nt in range(2):
                    po = ps1.tile([P, 512], F32, tag="qkv_ps", name="po")
                    for jj in range(8):
                        nc.tensor.matmul(
                            po[:],
                            ctxs[:, jj, :],
                            wo[:, jj, nt * 512:(nt + 1) * 512],
                            start=(jj == 0), stop=(jj == 7),
                        )
                    ob = p3.tile([P, 512], F32, tag="ob", name="ob", bufs=3)
                    nc.vector.tensor_copy(ob[:], po[:])
                    nc.sync.dma_start(
                        out_d[r0:r0 + P, nt * 512:(nt + 1) * 512], ob[:])

            # ---- emission schedule ----
            st00 = {}
            xts[(0, 0)] = xt00
            qkv_m(0, 0, 0, xt00, st00)
            xts[(0, 1)] = load_xt(0, 1)
            qkv_m(0, 0, 1, xt00, st00)
            vt00 = qkv_m(0, 0, 2, xt00, st00)
            nc.sync.dma_start(
                wo[:], woutT_d[:].rearrange("(j p) n -> p j n", p=P))

            def u_load(b, ch):
                return lambda: xts.__setitem__((b, ch), load_xt(b, ch))

            # each chunk's xt load is pumped ~4 units (one chunk) ahead
            chunks = [(0, 1), (0, 2), (0, 3), (1, 0), (1, 1), (1, 2), (1, 3)]
            fillers.append(lambda: v_transposes(0, 0, vt00))
            for i, (b, ch) in enumerate(chunks):
                if i + 1 < len(chunks):
                    fillers.append(u_load(*chunks[i + 1]))
                fillers.extend(p1_units(b, ch))
            n_units = len(fillers)  # 35

            for qs in range(4):
                p2_qs(0, qs)
                # chunk qs+1 of batch 0 must be fully emitted before its
                # attention chunk (scores need qT/kT, PV needs V)
                if qs < 3:
                    while n_units - len(fillers) < 1 + 5 * (qs + 1):
                        pump()
                if qs == 1:
                    fillers.append(lambda: ctxs_load(0, 0))
            drain()  # finish all of batch 1's QKV before its attention
            fillers.append(lambda: out_proj(0, 0))
            fillers.append(lambda: ctxs_load(0, 1))
            fillers.append(lambda: out_proj(0, 1))
            for qs in range(4):
                p2_qs(1, qs)
                if qs == 1:
                    drain()
                    fillers.append(lambda: ctxs_load(1, 0))
                if qs == 2:
                    fillers.append(lambda: out_proj(1, 0))
            drain()
            ctxs_load(1, 1)
            out_proj(1, 1)

    nc.finalize()
    return nc


_NC_CACHE = None


def _get_nc():
    global _NC_CACHE
    if _NC_CACHE is None:
        _NC_CACHE = _build_nc()
    return _NC_CACHE


def _host_tables():
    j = np.arange(32)
    inv = (10000.0 ** (-(j.astype(np.float64)) / 32.0))
    pos = np.arange(S, dtype=np.float64)
    fr = pos[:, None] * inv[None, :]              # [S, 32]
    import ml_dtypes
    cosT = np.cos(fr).T.astype(np.float32)        # [32, S]
    sinT = np.sin(fr).T.astype(np.float32)
    costab = np.tile(cosT, (4, 1)).astype(ml_dtypes.bfloat16)
    sintab = np.concatenate([-sinT, sinT, -sinT, sinT], 0).astype(
        ml_dtypes.bfloat16)
    kk = np.arange(P)[:, None]
    qq = np.arange(P)[None, :]
    mask1 = np.where(kk <= qq, 1.0, 0.0)
    maskT = np.concatenate([mask1, mask1], axis=1).astype(ml_dtypes.bfloat16)
    identf = np.eye(P, dtype=np.float32).astype(ml_dtypes.bfloat16)
    return costab, sintab, maskT, identf


def _make_in_maps(x, W_qkv, W_out):
    import ml_dtypes
    costab, sintab, maskT, identf = _host_tables()
    xT = np.ascontiguousarray(x.reshape(T, H).T).astype(ml_dtypes.bfloat16)
    woutT = np.ascontiguousarray(W_out.T).astype(ml_dtypes.bfloat16)
    in_maps = []
    for c in range(NCORES):
        h0 = 2 * c
        rows = np.concatenate([
            W_qkv[HD * h0:HD * (h0 + 2)],
            W_qkv[H + HD * h0:H + HD * (h0 + 2)],
            W_qkv[2 * H + HD * h0:2 * H + HD * (h0 + 2)],
        ], axis=0)                                        # [384, H]
        wqkvT = np.ascontiguousarray(rows.T).astype(ml_dtypes.bfloat16)
        in_maps.append({
            "xT": xT, "wqkvT": wqkvT, "woutT": woutT,
            "costab": costab, "sintab": sintab,
            "maskT": maskT, "identf": identf,
        })
    return in_maps


def _run_spmd(x, W_qkv, W_out, **kw):
    nc = _get_nc()
    in_maps = _make_in_maps(x, W_qkv, W_out)
    return run_bass_kernel_spmd(nc, in_maps, core_ids=list(range(NCORES)),
                                **kw)


def kernel(x, W_qkv, W_out):
    x = np.asarray(x, dtype=np.float32)
    W_qkv = np.asarray(W_qkv, dtype=np.float32)
    W_out = np.asarray(W_out, dtype=np.float32)
    res = _run_spmd(x, W_qkv, W_out)
    # core c owns, per batch b and half f, tokens [f*1024 + c*128, +128)
    full = np.empty((T, H), dtype=np.float32)
    for c in range(NCORES):
        o = res.results[c]["out"]
        for b in range(B):
            for f in range(2):
                t0 = b * S + f * 1024 + c * P
                full[t0:t0 + P] = o[b * 256 + f * P:b * 256 + (f + 1) * P]
    return full.reshape(B, S, H)


# revision 22
# speedup vs baseline: 1.1477x; 1.0240x over previous
"""Multi-head causal attention (B=2, S=2048, H=1024, 16 heads x 64, RoPE) on 8 trn2 cores.

Sharding: tensor-parallel over heads (2 heads/core) for QKV+attention, then
AllToAlls switch to token-parallel for the output projection. Each core owns
4x128-token slices (one per batch-half); the host concatenates row slices.

Key structure (per core c, heads h0=2c, h0+1):
 - xT [1024, 4096] feature-major bf16 activations (host-transposed), one DMA
   per 512-token chunk. QKV = 24 matmuls/chunk from a [128, 8, 384] W tile.
 - RoPE feature-major in bf16 (2x DVE modes) with host cos/sin tables; the
   half-rotation partition swap is 4 SBUF->SBUF DMAs from a scratch tile
   (no WAR hazard), sin-mul on GpSimd, cos-mul + add on DVE.
 - Scores TRANSPOSED in bf16: sT[k, q] = matmul(lhsT=kT_blk, rhs=qT_chunk),
   both heads in one [128, 2, 512] PSUM tile -> ONE merged exp per k-block.
   Softmax max-subtraction skipped (logits ~N(0,1)). Causal mask = bf16 0/1
   multiply on the diagonal block.
 - PV FLIPPED: ctx[q, d] = matmul(lhsT=probsT[k, q-blk], rhs=[V | ones]).
   Cost follows the 65-wide free dim, and the ones column accumulates the
   softmax sums for free (column 64).
 - Normalize: sums are a per-partition scalar -> DVE reciprocal + GpSimd
   tensor_scalar multiplies; ctx -> ctxT via PE transpose + DVE copy.
 - FOUR collectives (one per batch-half, [8, 128, 128] bf16) so only the
   last 256KB AllToAll is exposed in the tail; out-projections for earlier
   halves run as soon as their collective lands.
 - Emission uses a filler pump: phase-1 QKV m-groups / V-transposes /
   out-projections are interleaved between attention k-block units so the
   PE never idles (and stays p-state ramped) while ACT streams exps.
"""

from collections import deque

import numpy as np

import concourse.bacc as bacc
import concourse.mybir as mybir
import concourse.tile as tile
from concourse.bass_utils import run_bass_kernel_spmd

F32 = mybir.dt.float32
BF16 = mybir.dt.bfloat16
EXP = mybir.ActivationFunctionType.Exp

B, S, H = 2, 2048, 1024
NH, HD = 16, 64
NCORES = 8
T = B * S            # 4096 flattened tokens (b-major)
TBLK = T // NCORES   # 512 tokens per core
P = 128


def _build_nc():
    nc = bacc.Bacc(None, num_devices=NCORES)

    xT_d = nc.dram_tensor("xT", [H, T], BF16, kind="ExternalInput")
    wqkvT_d = nc.dram_tensor("wqkvT", [H, 384], BF16, kind="ExternalInput")
    woutT_d = nc.dram_tensor("woutT", [H, H], BF16, kind="ExternalInput")
    costab_d = nc.dram_tensor("costab", [P, S], BF16, kind="ExternalInput")
    sintab_d = nc.dram_tensor("sintab", [P, S], BF16, kind="ExternalInput")
    maskT_d = nc.dram_tensor("maskT", [P, 2 * P], BF16, kind="ExternalInput")
    identf_d = nc.dram_tensor("identf", [P, P], BF16, kind="ExternalInput")
    out_d = nc.dram_tensor("out", [TBLK, H], F32, kind="ExternalOutput")

    with tile.TileContext(nc) as tc:
        with (
            tc.tile_pool(name="long", bufs=1) as lp,
            tc.tile_pool(name="dram", bufs=1, space="DRAM") as dp,
            tc.tile_pool(name="p1s", bufs=4) as p1s,
            tc.tile_pool(name="p1v", bufs=2) as p1v,
            tc.tile_pool(name="p1t", bufs=3) as p1t,
            tc.tile_pool(name="ps1", bufs=1, space="PSUM") as ps1,
            tc.tile_pool(name="ps1t", bufs=1, space="PSUM") as ps1t,
            tc.tile_pool(name="p2", bufs=8) as p2,
            tc.tile_pool(name="p2n", bufs=3) as p2n,
            tc.tile_pool(name="ps2s", bufs=2, space="PSUM") as ps2s,
            tc.tile_pool(name="ps2c", bufs=1, space="PSUM") as ps2c,
            tc.tile_pool(name="p3", bufs=2) as p3,
        ):
            # long-lived tiles
            qT = [lp.tile([P, S], BF16, tag=f"qT{b}", name=f"qT{b}") for b in range(B)]
            kT = [lp.tile([P, S], BF16, tag=f"kT{b}", name=f"kT{b}") for b in range(B)]
            V = [[lp.tile([P, 16, 65], BF16, tag=f"V{b}{h}", name=f"V{b}{h}")
                  for h in range(2)] for b in range(B)]
            ctxT = [lp.tile([P, S], BF16, tag=f"ctxT{b}", name=f"ctxT{b}")
                    for b in range(B)]
            maskT_t = lp.tile([P, 2, P], BF16, tag="maskT")
            identf_t = lp.tile([P, P], BF16, tag="identf")
            wo = lp.tile([P, 8, H], BF16, tag="wo")
            wq = lp.tile([P, 8, 384], BF16, tag="wq")
            costab_t = lp.tile([P, S], BF16, tag="costab")
            sintab_t = lp.tile([P, S], BF16, tag="sintab")

            nc.sync.dma_start(wq[:], wqkvT_d[:].rearrange("(k p) c -> p k c", p=P))

            def load_xt(b, ch):
                tok0 = b * S + ch * 512
                xt = p1s.tile([P, 8, 512], BF16, tag="xt", name=f"xt{b}{ch}")
                nc.sync.dma_start(
                    xt[:], xT_d[:, tok0:tok0 + 512]
                    .rearrange("(k p) t -> p k t", p=P))
                return xt

            xt00 = load_xt(0, 0)
            nc.sync.dma_start(
                maskT_t[:], maskT_d[:].rearrange("p (h k) -> p h k", h=2))
            nc.sync.dma_start(identf_t[:], identf_d[:])
            nc.sync.dma_start(costab_t[:], costab_d[:])
            nc.sync.dma_start(sintab_t[:], sintab_d[:])
            for b in range(B):
                for h in range(2):
                    nc.vector.memset(V[b][h][:, :, 64:65], 1.0)

            a2a_in = [[dp.tile([NCORES, P, P], BF16, name=f"a2a_in{b}{f}",
                               tag=f"a2a_in{b}{f}") for f in range(2)]
                      for b in range(B)]
            a2a_out = [[dp.tile([NCORES, P, P], BF16, name=f"a2a_out{b}{f}",
                                tag=f"a2a_out{b}{f}") for f in range(2)]
                       for b in range(B)]

            def qkv_m(b, ch, m, xt, st):
                """One QKV output-tile: 8 matmuls + RoPE or V staging.
                q and k stage into one [P, 2, 512] tmp so the partition-swap
                is 4 chunk-level DMAs instead of 8."""
                c0 = ch * 512
                ps = ps1.tile([P, 512], F32, tag="qkv_ps")
                for kt in range(8):
                    nc.tensor.matmul(
                        ps[:], wq[:, kt, m * P:(m + 1) * P], xt[:, kt, :],
                        start=(kt == 0), stop=(kt == 7),
                    )
                if m < 2:
                    if m == 0:
                        st["tmp"] = p1t.tile([P, 2, 512], BF16, tag="tmp", name="tmp")
                        st["swp"] = p1t.tile([P, 2, 512], BF16, tag="swp", name="swp")
                    nc.vector.tensor_copy(st["tmp"][:, m, :], ps[:])
                    if m == 1:
                        tmp, swp = st["tmp"], st["swp"]
                        for g in range(4):
                            dst = g * 32
                            srcp = dst ^ 32
                            nc.sync.dma_start(swp[dst:dst + 32],
                                              tmp[srcp:srcp + 32])
                        for mm, tgt in ((0, qT[b]), (1, kT[b])):
                            nc.gpsimd.tensor_mul(
                                swp[:, mm, :], swp[:, mm, :],
                                sintab_t[:, c0:c0 + 512])
                            nc.vector.tensor_mul(
                                tgt[:, c0:c0 + 512], tmp[:, mm, :],
                                costab_t[:, c0:c0 + 512])
                            nc.vector.tensor_add(
                                tgt[:, c0:c0 + 512], tgt[:, c0:c0 + 512],
                                swp[:, mm, :])
                    return None
                vt = p1v.tile([P, 512], BF16, tag="vT", name=f"vT{b}{ch}")
                nc.vector.tensor_copy(vt[:], ps[:])
                return vt

            def v_transposes(b, ch, vt):
                for h in range(2):
                    pst = ps1t.tile([P, 256], BF16, tag="tp", name="vt_ps")
                    for tb in range(4):
                        nc.tensor.transpose(
                            pst[:, tb * HD:(tb + 1) * HD],
                            vt[h * HD:(h + 1) * HD, tb * P:(tb + 1) * P],
                            identf_t[h * HD:(h + 1) * HD, h * HD:(h + 1) * HD],
                        )
                    nc.vector.tensor_copy(
                        V[b][h][:, ch * 4:(ch + 1) * 4, 0:HD],
                        pst[:].rearrange("p (g d) -> p g d", g=4))

            xts = {}

            def p1_units(b, ch):
                """Filler units for one 512-token chunk of QKV+RoPE+V
                (the xt load for (b, ch) must be emitted beforehand)."""
                state = {"vt": None, "st": {}}

                def u_m(m):
                    r = qkv_m(b, ch, m, xts[(b, ch)], state["st"])
                    if r is not None:
                        state["vt"] = r

                def u_vt():
                    v_transposes(b, ch, state["vt"])

                units = [lambda m=m: u_m(m) for m in range(3)]
                units.append(u_vt)
                return units

            fillers = deque()

            def pump():
                if fillers:
                    fillers.popleft()()

            def drain(n=None):
                cnt = len(fillers) if n is None else n
                for _ in range(cnt):
                    pump()

            def _pv_group(pctx, b, qs, qb, pbs_all):
                """All of query-block qb's PV accumulation as one contiguous
                PSUM group (banks allow only one open group at a time)."""
                last = 4 * qs + qb
                for h in range(2):
                    for kb in range(last + 1):
                        nc.tensor.matmul(
                            pctx[:, 2 * qb + h, 0:65],
                            pbs_all[kb][:, h, qb * P:(qb + 1) * P],
                            V[b][h][:, kb, :],
                            start=(kb == 0), stop=(kb == last),
                            skip_group_check=True,
                        )

            def p2_qs(b, qs):
                """Attention for one 512-query chunk: scores, exp, flipped PV,
                normalize, transpose back to ctxT. Pumps one filler unit per
                k-block to keep the PE busy while ACT runs the exps."""
                nkb = 4 * qs + 4
                pctx = ps2c.tile([P, 8, P], F32, tag="ctx", name="pctx")
                rb = p2n.tile([P, 4, 2, 1], F32, tag="recip")

                def normalize(qb):
                    # region qb of pctx just received its last accumulation
                    nc.vector.reciprocal(
                        rb[:, qb, :, :], pctx[:, 2 * qb:2 * qb + 2, 64:65])
                    cs = p2n.tile([P, 2, HD], BF16, tag="csb", bufs=4)
                    for h in range(2):
                        nc.vector.tensor_scalar_mul(
                            cs[:, h, :], pctx[:, 2 * qb + h, 0:HD],
                            rb[:, qb, h, 0:1])
                    pt = ps1t.tile([P, 256], BF16, tag="tp", name="ctxt_ps")
                    nc.tensor.transpose(pt[:, 0:P], cs[:], identf_t[:])
                    q0 = qs * 512 + qb * P
                    nc.vector.tensor_copy(ctxT[b][:, q0:q0 + P], pt[:, 0:P])

                pbs_all = []
                for kb in range(nkb):
                    j = kb - 4 * qs
                    qoff = max(0, j) * P
                    psT = ps2s.tile([P, 2, 512], F32, tag="sT")
                    for h in range(2):
                        nc.tensor.matmul(
                            psT[:, h, qoff:512],
                            kT[b][h * HD:(h + 1) * HD, kb * P:(kb + 1) * P],
                            qT[b][h * HD:(h + 1) * HD,
                                  qs * 512 + qoff:(qs + 1) * 512],
                            start=True, stop=True,
                            tile_position=(h * HD, 0),
                            skip_group_check=True,
                        )
                    pb = p2.tile([P, 2, 512], BF16, tag="probs", bufs=18)
                    nc.scalar.activation(
                        pb[:, :, qoff:512], psT[:, :, qoff:512], EXP,
                        scale=0.125)
                    if j >= 0:
                        nc.vector.tensor_mul(
                            pb[:, :, qoff:qoff + P],
                            pb[:, :, qoff:qoff + P], maskT_t[:])
                    pbs_all.append(pb)
                    pump()
                    if j >= 0:
                        # query-block j's last k-block just got its probs:
                        # emit its full PV accumulation + normalize
                        _pv_group(pctx, b, qs, j, pbs_all)
                        normalize(j)
                if qs % 2 == 1:
                    half = qs // 2
                    nc.sync.dma_start(
                        a2a_in[b][half][:].rearrange("g p t -> p g t"),
                        ctxT[b][:, half * 1024:(half + 1) * 1024]
                        .rearrange("p (g t) -> p g t", g=8))
                    nc.gpsimd.collective_compute(
                        "AllToAll",
                        mybir.AluOpType.bypass,
                        replica_groups=[list(range(NCORES))],
                        ins=[a2a_in[b][half].opt()],
                        outs=[a2a_out[b][half].opt()],
                    )

            ctxs_t = {}

            def ctxs_load(bb, half):
                ctxs = p3.tile([P, 8, P], BF16, tag="ctxs",
                               name=f"ctxs{bb}{half}")
                nc.sync.dma_start(
                    ctxs[:], a2a_out[bb][half][:].rearrange("j p t -> p j t"))
                ctxs_t[(bb, half)] = ctxs

            def out_proj(bb, half):
                """Out-projection for this core's 128 tokens of one
                batch-half; lands in out_d rows [bb*256+half*128, +128)."""
                ctxs = ctxs_t[(bb, half)]
                r0 = bb * 256 + half * P
                for nt in range(2):
                    po = ps1.tile([P, 512], F32, tag="qkv_ps", name="po")
                    for jj in range(8):
                        nc.tensor.matmul(
                            po[:],
                            ctxs[:, jj, :],
                            wo[:, jj, nt * 512:(nt + 1) * 512],
                            start=(jj == 0), stop=(jj == 7),
                        )
                    ob = p3.tile([P, 512], F32, tag="ob", name="ob", bufs=3)
                    nc.vector.tensor_copy(ob[:], po[:])
                    nc.sync.dma_start(
                        out_d[r0:r0 + P, nt * 512:(nt + 1) * 512], ob[:])

            # ---- emission schedule ----
            st00 = {}
            xts[(0, 0)] = xt00
            qkv_m(0, 0, 0, xt00, st00)
            xts[(0, 1)] = load_xt(0, 1)
            qkv_m(0, 0, 1, xt00, st00)
            vt00 = qkv_m(0, 0, 2, xt00, st00)
            nc.sync.dma_start(
                wo[:], woutT_d[:].rearrange("(j p) n -> p j n", p=P))

            def u_load(b, ch):
                return lambda: xts.__setitem__((b, ch), load_xt(b, ch))

            # each chunk's xt load is pumped ~4 units (one chunk) ahead
            chunks = [(0, 1), (0, 2), (0, 3), (1, 0), (1, 1), (1, 2), (1, 3)]
            fillers.append(lambda: v_transposes(0, 0, vt00))
            for i, (b, ch) in enumerate(chunks):
                if i + 1 < len(chunks):
                    fillers.append(u_load(*chunks[i + 1]))
                fillers.extend(p1_units(b, ch))
            n_units = len(fillers)  # 35

            for qs in range(4):
                p2_qs(0, qs)
                # chunk qs+1 of batch 0 must be fully emitted before its
                # attention chunk (scores need qT/kT, PV needs V)
                if qs < 3:
                    while n_units - len(fillers) < 1 + 5 * (qs + 1):
                        pump()
                if qs == 1:
                    fillers.append(lambda: ctxs_load(0, 0))
            drain()  # finish all of batch 1's QKV before its attention
            fillers.append(lambda: out_proj(0, 0))
            fillers.append(lambda: ctxs_load(0, 1))
            fillers.append(lambda: out_proj(0, 1))
            for qs in range(4):
                p2_qs(1, qs)
                if qs == 1:
                    drain()
                    fillers.append(lambda: ctxs_load(1, 0))
                if qs == 2:
                    fillers.append(lambda: out_proj(1, 0))
            drain()
            ctxs_load(1, 1)
            out_proj(1, 1)

    nc.finalize()
    return nc


_NC_CACHE = None


def _get_nc():
    global _NC_CACHE
    if _NC_CACHE is None:
        _NC_CACHE = _build_nc()
    return _NC_CACHE


def _host_tables():
    j = np.arange(32)
    inv = (10000.0 ** (-(j.astype(np.float64)) / 32.0))
    pos = np.arange(S, dtype=np.float64)
    fr = pos[:, None] * inv[None, :]              # [S, 32]
    import ml_dtypes
    cosT = np.cos(fr).T.astype(np.float32)        # [32, S]
    sinT = np.sin(fr).T.astype(np.float32)
    costab = np.tile(cosT, (4, 1)).astype(ml_dtypes.bfloat16)
    sintab = np.concatenate([-sinT, sinT, -sinT, sinT], 0).astype(
        ml_dtypes.bfloat16)
    kk = np.arange(P)[:, None]
    qq = np.arange(P)[None, :]
    mask1 = np.where(kk <= qq, 1.0, 0.0)
    maskT = np.concatenate([mask1, mask1], axis=1).astype(ml_dtypes.bfloat16)
    identf = np.eye(P, dtype=np.float32).astype(ml_dtypes.bfloat16)
    return costab, sintab, maskT, identf


def _make_in_maps(x, W_qkv, W_out):
    import ml_dtypes
    costab, sintab, maskT, identf = _host_tables()
    xT = np.ascontiguousarray(x.reshape(T, H).T).astype(ml_dtypes.bfloat16)
    woutT = np.ascontiguousarray(W_out.T).astype(ml_dtypes.bfloat16)
    in_maps = []
    for c in range(NCORES):
        h0 = 2 * c
        rows = np.concatenate([
            W_qkv[HD * h0:HD * (h0 + 2)],
            W_qkv[H + HD * h0:H + HD * (h0 + 2)],
            W_qkv[2 * H + HD * h0:2 * H + HD * (h0 + 2)],
        ], axis=0)                                        # [384, H]
        wqkvT = np.ascontiguousarray(rows.T).astype(ml_dtypes.bfloat16)
        in_maps.append({
            "xT": xT, "wqkvT": wqkvT, "woutT": woutT,
            "costab": costab, "sintab": sintab,
            "maskT": maskT, "identf": identf,
        })
    return in_maps


def _run_spmd(x, W_qkv, W_out, **kw):
    nc = _get_nc()
    in_maps = _make_in_maps(x, W_qkv, W_out)
    return run_bass_kernel_spmd(nc, in_maps, core_ids=list(range(NCORES)),
                                **kw)


def kernel(x, W_qkv, W_out):
    x = np.asarray(x, dtype=np.float32)
    W_qkv = np.asarray(W_qkv, dtype=np.float32)
    W_out = np.asarray(W_out, dtype=np.float32)
    res = _run_spmd(x, W_qkv, W_out)
    # core c owns, per batch b and half f, tokens [f*1024 + c*128, +128)
    full = np.empty((T, H), dtype=np.float32)
    for c in range(NCORES):
        o = res.results[c]["out"]
        for b in range(B):
            for f in range(2):
                t0 = b * S + f * 1024 + c * P
                full[t0:t0 + P] = o[b * 256 + f * P:b * 256 + (f + 1) * P]
    return full.reshape(B, S, H)
